# revision 14
# baseline (speedup 1.0000x reference)
"""MLA (CustomLlamaMLAForInfer) Trainium2 Bass kernel, v2.

Sharding: tensor-parallel over heads across 8 NeuronCores. Core c owns
kv-head c and q-heads [4c, 4c+4). Every core sees the full token stream
(B*S = 4096 tokens); o_proj is computed against the core's 512
head-dims, producing a partial [4096, 4096] bf16 output that the host
sums across the 8 cores.

v2 changes vs baseline:
  - Host fuses Wupk/Wupv through Wdk (k_c = hid @ (Wupk_c Wdk).T etc.),
    removing the replicated 512-dim latent projection and its DRAM
    round trip entirely.
  - Single phase-1 pass over hidT: one 6-bank PSUM group per token
    block produces q (4 tiles), interleaved k_rope/k_nope (1 tile,
    weight columns pre-permuted so no cross-partition moves at evict),
    vT (1 tile, PE-transposed to [tok, d]).
  - bf16 operands on the PE except p/v (f32r), halving DMA traffic.
  - qT stays resident in SBUF (no DRAM round trip).
  - Attention: scores for 2 k-tiles accumulate into one 2-bank PSUM
    tile, one wide exp (N=1024) per group; softmax denominators via
    ones-matmul; reciprocal_approx_fast instead of iterative reciprocal.
  - o_proj interleaved per (b, qb) block right after its 4 heads
    finish, sharing PSUM banks with the scores pool; qb descending so
    the wo prefetch hides under the deepest attention block.
"""

import numpy as np

HIDDEN = 4096
N_HEADS = 32
KV_HEADS = 8
HEAD_DIM = 128
LOW_RANK = 64
TOP_K_ROPE = 32
ROPE_THETA = 10000.0
B, S = 2, 2048
NCORES = 8
HPC = N_HEADS // NCORES          # q heads per core = 4
QR = HPC * HEAD_DIM              # q rows per core = 512
CD = LOW_RANK * KV_HEADS         # latent dim = 512
KRR = 2 * TOP_K_ROPE             # rope rows per kv head = 64
WKV = 256                        # fused kv out rows: kr 64 + kc 64 + v 128
WC = QR + WKV                    # combined projection out rows = 768


def _rope_tables(seq_len):
    inv = 1.0 / (ROPE_THETA ** (np.arange(0, HEAD_DIM, 2, dtype=np.float32) / HEAD_DIM))
    pos = np.arange(seq_len, dtype=np.float32)
    fr = np.outer(pos, inv)
    emb = np.concatenate([fr, fr], axis=-1)          # [S, 128]
    return (np.cos(emb).T.astype(np.float32),        # [128, S]
            np.sin(emb).T.astype(np.float32))


def build_program(Bv=B, Sv=S, TB=512, QB=512, trace_sim=False):
    from concourse import bacc, tile, mybir
    import concourse.bass as bass

    f32 = mybir.dt.float32
    F32R = mybir.dt.float32r
    BF16 = mybir.dt.bfloat16
    FP8 = mybir.dt.float8e4
    DR = mybir.MatmulPerfMode.DoubleRow
    MS = bass.MemorySpace
    EXP = mybir.ActivationFunctionType.Exp

    NT = Bv * Sv                 # total tokens = 4096
    HT = HIDDEN // 128           # hidden tiles = 32
    NTB = NT // TB               # proj token blocks = 8
    NQB = Sv // QB               # q blocks per batch = 4
    NJ = QB // 128               # diagonal mask variants = 4
    NKT_B = Sv // 128            # k tiles per batch = 16
    QT = QR // 128               # q-head tiles per core = 4

    nc = bacc.Bacc("TRN2", target_bir_lowering=False, debug=False,
                   num_devices=NCORES)

    def din(name, shape, dt=BF16):
        return nc.dram_tensor(name, shape, dt, kind="ExternalInput").ap()

    hidT = din("hidT", [HIDDEN, NT])
    wcomb = din("wcomb", [HIDDEN, WC])
    wo = din("wo_t", [QR, HIDDEN])
    ropes = din("ropes", [128, 4, NT])   # 0=qcos 1=qsin 2=kcos 3=ksin
    masks = din("masks", [128, NJ, QB], f32)
    onesd = din("ones", [128, 1], f32)
    identd = din("ident", [128, 128], f32)
    outp = nc.dram_tensor("out_part", [NT, HIDDEN], BF16, kind="ExternalOutput").ap()

    with tile.TileContext(nc, trace_sim=trace_sim) as tc:
        with tc.tile_pool(name="persist", bufs=1) as pers:
            kT = pers.tile([128, NT], BF16, tag="kT")
            qT = pers.tile([128, QT, NT], BF16, tag="qT")
            v_tok = pers.tile([128, NT // 128, HEAD_DIM], F32R, tag="vtok")

            # ---------------- phase 1: fused projections of hidden ----------
            with tc.tile_pool(name="p1c", bufs=1) as cp, \
                 tc.tile_pool(name="hid", bufs=4) as hp, \
                 tc.tile_pool(name="rps", bufs=2) as rpp, \
                 tc.tile_pool(name="st1", bufs=2) as st, \
                 tc.tile_pool(name="ps1", bufs=6, space=MS.PSUM) as pp, \
                 tc.tile_pool(name="psT", bufs=2, space=MS.PSUM) as pvt:
                ident_sb = cp.tile([128, 128], F32R, tag="id")
                nc.scalar.dma_start(ident_sb[:], identd.bitcast(F32R))
                wc_sb = cp.tile([128, HT, WC], BF16, tag="wc")
                wc_r = wcomb.rearrange("(t p) w -> p t w", p=128)
                for qtr in range(4):
                    t0, t1 = qtr * (HT // 4), (qtr + 1) * (HT // 4)
                    nc.scalar.dma_start(wc_sb[:, t0:t1, :], wc_r[:, t0:t1, :])

                for blk in range(NTB):
                    c0, c1 = blk * TB, (blk + 1) * TB
                    rp = rpp.tile([128, 4, TB], BF16, tag="rp")
                    nc.sync.dma_start(rp[:], ropes[:, :, c0:c1])
                    hts = []
                    for half in range(2):
                        ht = hp.tile([128, HT // 2, TB], BF16, tag="hid")
                        if blk == 0:
                            for q4 in range(2):
                                nc.sync.dma_start(
                                    ht[:, q4 * 8:(q4 + 1) * 8, :],
                                    hidT[half * 2048 + q4 * 1024:
                                         half * 2048 + (q4 + 1) * 1024, c0:c1]
                                    .rearrange("(t p) w -> p t w", p=128))
                        else:
                            nc.sync.dma_start(
                                ht[:],
                                hidT[half * 2048:(half + 1) * 2048, c0:c1]
                                .rearrange("(t p) w -> p t w", p=128))
                        hts.append(ht)
                    ps = [pp.tile([128, TB], f32, tag="ps1", name=f"ps{_m}")
                          for _m in range(6)]
                    for t in range(HT):
                        htt = hts[t // 16][:, t % 16, :]
                        for m in range(6):
                            nc.tensor.matmul(
                                ps[m][:],
                                wc_sb[:, t, m * 128:(m + 1) * 128],
                                htt,
                                start=(t == 0), stop=(t == HT - 1))
                    # ---- evict q tiles (rope via sign-folded tables) ----
                    for m in range(QT):
                        qraw = st.tile([128, TB], BF16, tag="qraw")
                        nc.scalar.copy(qraw[:], ps[m][:])
                        rot = st.tile([128, TB], BF16, tag="rot")
                        nc.sync.dma_start(rot[0:64, :], qraw[64:128, :])
                        nc.sync.dma_start(rot[64:128, :], qraw[0:64, :])
                        t1 = st.tile([128, TB], BF16, tag="t1")
                        nc.vector.tensor_mul(t1[:], qraw[:], rp[:, 0, :])
                        nc.vector.tensor_mul(rot[:], rot[:], rp[:, 1, :])
                        nc.vector.tensor_add(qT[:, m, c0:c1], t1[:], rot[:])
                    # ---- evict kv tile: rows 0:32 krA, 32:64 kcA,
                    #      64:96 krB, 96:128 kcB (host pre-permuted) ----
                    kk = st.tile([128, TB], BF16, tag="kk")
                    nc.scalar.copy(kk[:], ps[4][:])
                    rt = st.tile([128, TB], BF16, tag="rt")
                    nc.sync.dma_start(rt[0:32, :], kk[64:96, :])
                    nc.sync.dma_start(rt[64:96, :], kk[0:32, :])
                    t2 = st.tile([128, TB], BF16, tag="t2")
                    for r0 in (0, 64):
                        nc.vector.tensor_mul(
                            t2[r0:r0 + 32, :], kk[r0:r0 + 32, :],
                            rp[r0:r0 + 32, 2, :])
                        nc.vector.tensor_mul(
                            rt[r0:r0 + 32, :], rt[r0:r0 + 32, :],
                            rp[r0:r0 + 32, 3, :])
                        nc.vector.tensor_add(
                            kT[r0:r0 + 32, c0:c1], t2[r0:r0 + 32, :],
                            rt[r0:r0 + 32, :])
                    nc.scalar.copy(kT[32:64, c0:c1], kk[32:64, :])
                    nc.scalar.copy(kT[96:128, c0:c1], kk[96:128, :])
                    # ---- evict v tile: vT [d, tok] -> transpose to [tok, d] --
                    vst = st.tile([128, TB], F32R, tag="vst")
                    nc.scalar.copy(vst[:], ps[5][:])
                    for tt in range(TB // 128):
                        vtp = pvt.tile([128, 128], F32R, tag="vtp")
                        nc.tensor.transpose(
                            vtp[:], vst[:, tt * 128:(tt + 1) * 128], ident_sb[:])
                        nc.vector.tensor_copy(
                            v_tok[:, blk * (TB // 128) + tt, :], vtp[:])

            # ---------------- phases 3+4 interleaved ----------------
            with tc.tile_pool(name="wop", bufs=1) as wop, \
                 tc.tile_pool(name="attn", bufs=1) as ap_, \
                 tc.tile_pool(name="c3", bufs=1) as cst3, \
                 tc.tile_pool(name="pt", bufs=4) as ptp, \
                 tc.tile_pool(name="pf", bufs=3) as pfp, \
                 tc.tile_pool(name="sm", bufs=2) as smp, \
                 tc.tile_pool(name="st4", bufs=2) as st4, \
                 tc.tile_pool(name="psS", bufs=2, space=MS.PSUM) as psS, \
                 tc.tile_pool(name="psO", bufs=2, space=MS.PSUM) as psO, \
                 tc.tile_pool(name="psU", bufs=2, space=MS.PSUM) as psU:
                masks_sb = cst3.tile([128, NJ, QB], F32R, tag="masks")
                nc.scalar.dma_start(masks_sb[:], masks.bitcast(F32R))
                ones_sb = cst3.tile([128, 1], F32R, tag="ones")
                nc.scalar.dma_start(ones_sb[:], onesd.bitcast(F32R))
                wo_sb = wop.tile([128, QT, HIDDEN], BF16, tag="wo")
                nc.scalar.dma_start(wo_sb[:], wo.rearrange("(t p) w -> p t w", p=128))
                attn_sb = ap_.tile([128, QT, NT], BF16, tag="attn")

                def emit_attn(b, qb, filler):
                    off = b * Sv
                    q0 = off + qb * QB
                    ngrp = NJ * (qb + 1) // 2   # groups of 2 k-tiles
                    # spread the previous block's o_proj units evenly over
                    # all drain slots (after each group's scores + at each
                    # h boundary, where the exp-latency bubbles sit)
                    nfill = 33
                    slots = QT * (ngrp + 1)
                    base, rem = divmod(nfill, slots)
                    slot = [0]

                    def drain():
                        k = base + (1 if slot[0] < rem else 0)
                        slot[0] += 1
                        for _ in range(k):
                            next(filler, None)
                    for h in range(QT):
                        ops = psO.tile([128, QB], f32, tag="ops")
                        sps = psU.tile([1, QB], f32, tag="sps")
                        for g in range(ngrp):
                            scp = psS.tile([128, 2, QB], f32, tag="scp")
                            for j2 in range(2):
                                kt = 2 * g + j2
                                j = kt - NJ * qb        # diag idx if >= 0
                                tr = j * 128 if j > 0 else 0  # trim cols
                                nc.tensor.matmul(
                                    scp[:, j2, tr:QB],
                                    kT[:, off + kt * 128: off + (kt + 1) * 128],
                                    qT[:, h, q0 + tr:q0 + QB],
                                    start=True, stop=True)
                            # o_proj filler: fills the PE bubble while the
                            # activation engine computes exp of this group
                            drain()
                            ptile = ptp.tile([128, 2, QB], F32R, tag="pt")
                            nc.scalar.activation(ptile[:], scp[:], EXP)
                            dg = g - NJ * qb // 2
                            if dg >= 0:   # diagonal groups: causal mask
                                nc.vector.tensor_mul(
                                    ptile[:], ptile[:],
                                    masks_sb[:, 2 * dg:2 * dg + 2, :])
                            for j2 in range(2):
                                kt = 2 * g + j2
                                j = kt - NJ * qb
                                tr = j * 128 if j > 0 else 0
                                first = (g == 0 and j2 == 0)
                                last = (g == ngrp - 1 and j2 == 1)
                                nc.tensor.matmul(
                                    ops[:, tr:QB],
                                    v_tok[:, b * NKT_B + kt, :],
                                    ptile[:, j2, tr:QB],
                                    start=first, stop=last)
                                nc.tensor.matmul(
                                    sps[:, tr:QB],
                                    ones_sb[:],
                                    ptile[:, j2, tr:QB],
                                    start=first, stop=last)
                        rec = smp.tile([1, QB], f32, tag="rec")
                        nc.vector.reciprocal_approx_fast(rec[:], sps[:])
                        rb = smp.tile([128, QB], f32, tag="rb")
                        nc.gpsimd.partition_broadcast(rb[:], rec[:])
                        nc.vector.tensor_mul(
                            attn_sb[:, h, q0:q0 + QB], ops[:], rb[:])
                        drain()
                    # drain any leftover filler units
                    for _ in filler:
                        pass

                def oproj_units(b, qb):
                    """Generator: each next() emits one 4-matmul o_proj unit
                    (one 512-wide hidden chunk) for this token block."""
                    q0 = b * Sv + qb * QB
                    for Tt in range(QB // 128):
                        T0 = q0 + Tt * 128
                        orow = st4.tile([128, HIDDEN], BF16, tag="orow")
                        for npair in range(HIDDEN // 1024):
                            po = psS.tile([128, 2, QB], f32, tag="scp")
                            for half in range(2):
                                nn = 2 * npair + half
                                for h2 in range(QT):
                                    nc.tensor.matmul(
                                        po[:, half, :],
                                        attn_sb[:, h2, T0:T0 + 128],
                                        wo_sb[:, h2, nn * 512:(nn + 1) * 512],
                                        start=(h2 == 0), stop=(h2 == QT - 1))
                                yield
                            if npair % 2 == 0:
                                nc.vector.tensor_copy(
                                    orow[:, npair * 1024:(npair + 1) * 1024],
                                    po[:])
                            else:
                                nc.scalar.copy(
                                    orow[:, npair * 1024:(npair + 1) * 1024],
                                    po[:])
                        nc.sync.dma_start(outp[T0:T0 + 128, :], orow[:])
                    yield

                blocks = [(b, qb) for b in range(Bv)
                          for qb in range(NQB - 1, -1, -1)]
                prev = None
                for blk in blocks:
                    filler = oproj_units(*prev) if prev is not None else iter(())
                    emit_attn(*blk, filler)
                    prev = blk
                for _ in oproj_units(*prev):
                    pass

    nc.compile()
    return nc


def make_in_maps(hidden_states, Wq, Wkr, Wdk, Wupk, Wupv, Wo, Bv=B, Sv=S, QB=512):
    """Host-side sharding + layout prep. Returns per-core input dicts."""
    import ml_dtypes
    bf16 = ml_dtypes.bfloat16
    fp8 = ml_dtypes.float8_e4m3fn

    NT = Bv * Sv
    NJ = QB // 128
    scale = 1.0 / np.sqrt(np.float32(HEAD_DIM))

    hidden_states = np.asarray(hidden_states, dtype=np.float32)
    Wq = np.asarray(Wq, dtype=np.float32)
    Wkr = np.asarray(Wkr, dtype=np.float32)
    Wdk = np.asarray(Wdk, dtype=np.float32)
    Wupk = np.asarray(Wupk, dtype=np.float32)
    Wupv = np.asarray(Wupv, dtype=np.float32)
    Wo = np.asarray(Wo, dtype=np.float32)

    hidT = np.ascontiguousarray(
        hidden_states.reshape(NT, HIDDEN).T).astype(bf16)

    cos_t, sin_t = _rope_tables(Sv)                    # [128, S]
    cos_t = np.tile(cos_t, (1, Bv))                    # [128, NT]
    sin_t = np.tile(sin_t, (1, Bv))
    qcos = cos_t * scale
    qsin = np.concatenate([-sin_t[0:64], sin_t[64:128]], axis=0) * scale
    # k rope tables in the permuted kv-row layout:
    #   rows 0:32  = rotated rope dims 0:32   (krA):  cos[0:32],  -sin[0:32]
    #   rows 64:96 = rotated rope dims 64:96  (krB):  cos[64:96], +sin[64:96]
    kcos = np.zeros((128, NT), np.float32)
    ksin = np.zeros((128, NT), np.float32)
    kcos[0:32] = cos_t[0:32]
    kcos[64:96] = cos_t[64:96]
    ksin[0:32] = -sin_t[0:32]
    ksin[64:96] = sin_t[64:96]
    ropes = np.ascontiguousarray(
        np.stack([qcos, qsin, kcos, ksin], axis=1)).astype(bf16)  # [128,4,NT]

    k_idx = np.arange(128)[:, None]
    q_idx = np.arange(QB)[None, :]
    masks = np.stack(
        [(q_idx >= j * 128 + k_idx).astype(np.float32) for j in range(NJ)],
        axis=1)                                        # [128, NJ, QB]
    masks = np.ascontiguousarray(masks)

    in_maps = []
    for c in range(NCORES):
        wq_t = Wq[QR * c:QR * (c + 1)].T               # [HIDDEN, 512]
        wkr_c = Wkr[KRR * c:KRR * (c + 1)]             # [64, HIDDEN]
        wfk_c = Wupk[KRR * c:KRR * (c + 1)] @ Wdk      # [64, HIDDEN] fused
        wfv_c = Wupv[HEAD_DIM * c:HEAD_DIM * (c + 1)] @ Wdk  # [128, HIDDEN]
        kvrows = np.empty((128, HIDDEN), np.float32)
        kvrows[0:32] = wkr_c[0:32]     # krA: rope dims 0:32
        kvrows[32:64] = wfk_c[0:32]    # kcA: nope dims 32:64
        kvrows[64:96] = wkr_c[32:64]   # krB: rope dims 64:96
        kvrows[96:128] = wfk_c[32:64]  # kcB: nope dims 96:128
        wcomb = np.concatenate([wq_t, kvrows.T, wfv_c.T], axis=1)  # [HIDDEN, 768]
        wo_t = Wo[:, QR * c:QR * (c + 1)].T            # [512, HIDDEN]
        in_maps.append({
            "hidT": hidT,
            "wcomb": np.ascontiguousarray(wcomb).astype(bf16),
            "wo_t": np.ascontiguousarray(wo_t).astype(bf16),
            "ropes": ropes,
            "masks": masks,
            "ones": np.ones((128, 1), np.float32),
            "ident": np.eye(128, dtype=np.float32),
        })
    return in_maps


_NC_CACHE = {}


def _get_program(key=(B, S, 512, 512)):
    if key not in _NC_CACHE:
        _NC_CACHE[key] = build_program(*key)
    return _NC_CACHE[key]


def kernel(hidden_states, Wq, Wkr, Wdk, Wupk, Wupv, Wo):
    from concourse.bass_utils import run_bass_kernel_spmd

    in_maps = make_in_maps(np.asarray(hidden_states), np.asarray(Wq),
                           np.asarray(Wkr), np.asarray(Wdk), np.asarray(Wupk),
                           np.asarray(Wupv), np.asarray(Wo))
    nc = _get_program()
    res = run_bass_kernel_spmd(nc, in_maps, list(range(NCORES)))
    out = res.results[0]["out_part"].astype(np.float32)
    for i in range(1, NCORES):
        out = out + res.results[i]["out_part"].astype(np.float32)
    return out.reshape(B, S, HIDDEN).astype(np.float32)


# revision 15
# speedup vs baseline: 1.0274x; 1.0274x over previous
"""MLA (CustomLlamaMLAForInfer) Trainium2 Bass kernel, v2.

Sharding: tensor-parallel over heads across 8 NeuronCores. Core c owns
kv-head c and q-heads [4c, 4c+4). Every core sees the full token stream
(B*S = 4096 tokens); o_proj is computed against the core's 512
head-dims, producing a partial [4096, 4096] bf16 output that the host
sums across the 8 cores.

v2 changes vs baseline:
  - Host fuses Wupk/Wupv through Wdk (k_c = hid @ (Wupk_c Wdk).T etc.),
    removing the replicated 512-dim latent projection and its DRAM
    round trip entirely.
  - Single phase-1 pass over hidT: one 6-bank PSUM group per token
    block produces q (4 tiles), interleaved k_rope/k_nope (1 tile,
    weight columns pre-permuted so no cross-partition moves at evict),
    vT (1 tile, PE-transposed to [tok, d]).
  - bf16 operands on the PE except p/v (f32r), halving DMA traffic.
  - qT stays resident in SBUF (no DRAM round trip).
  - Attention: scores for 2 k-tiles accumulate into one 2-bank PSUM
    tile, one wide exp (N=1024) per group; softmax denominators via
    ones-matmul; reciprocal_approx_fast instead of iterative reciprocal.
  - o_proj interleaved per (b, qb) block right after its 4 heads
    finish, sharing PSUM banks with the scores pool; qb descending so
    the wo prefetch hides under the deepest attention block.
"""

import numpy as np

HIDDEN = 4096
N_HEADS = 32
KV_HEADS = 8
HEAD_DIM = 128
LOW_RANK = 64
TOP_K_ROPE = 32
ROPE_THETA = 10000.0
B, S = 2, 2048
NCORES = 8
HPC = N_HEADS // NCORES          # q heads per core = 4
QR = HPC * HEAD_DIM              # q rows per core = 512
CD = LOW_RANK * KV_HEADS         # latent dim = 512
KRR = 2 * TOP_K_ROPE             # rope rows per kv head = 64
WKV = 256                        # fused kv out rows: kr 64 + kc 64 + v 128
WC = QR + WKV                    # combined projection out rows = 768


def _rope_tables(seq_len):
    inv = 1.0 / (ROPE_THETA ** (np.arange(0, HEAD_DIM, 2, dtype=np.float32) / HEAD_DIM))
    pos = np.arange(seq_len, dtype=np.float32)
    fr = np.outer(pos, inv)
    emb = np.concatenate([fr, fr], axis=-1)          # [S, 128]
    return (np.cos(emb).T.astype(np.float32),        # [128, S]
            np.sin(emb).T.astype(np.float32))


def build_program(Bv=B, Sv=S, TB=512, QB=512, trace_sim=False):
    from concourse import bacc, tile, mybir
    import concourse.bass as bass

    f32 = mybir.dt.float32
    F32R = mybir.dt.float32r
    BF16 = mybir.dt.bfloat16
    FP8 = mybir.dt.float8e4
    DR = mybir.MatmulPerfMode.DoubleRow
    MS = bass.MemorySpace
    EXP = mybir.ActivationFunctionType.Exp

    NT = Bv * Sv                 # total tokens = 4096
    HT = HIDDEN // 128           # hidden tiles = 32
    NTB = NT // TB               # proj token blocks = 8
    NQB = Sv // QB               # q blocks per batch = 4
    NJ = QB // 128               # diagonal mask variants = 4
    NKT_B = Sv // 128            # k tiles per batch = 16
    QT = QR // 128               # q-head tiles per core = 4

    nc = bacc.Bacc("TRN2", target_bir_lowering=False, debug=False,
                   num_devices=NCORES)

    def din(name, shape, dt=BF16):
        return nc.dram_tensor(name, shape, dt, kind="ExternalInput").ap()

    hidT = din("hidT", [HIDDEN, NT])
    wcomb = din("wcomb", [HIDDEN, WC])
    wo = din("wo_t", [QR, HIDDEN])
    ropes = din("ropes", [128, 4, NT])   # 0=qcos 1=qsin 2=kcos 3=ksin
    masks = din("masks", [128, NJ, QB], f32)
    onesd = din("ones", [128, 1], f32)
    identd = din("ident", [128, 128], f32)
    outp = nc.dram_tensor("out_part", [NT, HIDDEN], BF16, kind="ExternalOutput").ap()

    with tile.TileContext(nc, trace_sim=trace_sim) as tc:
        with tc.tile_pool(name="persist", bufs=1) as pers:
            kT = pers.tile([128, NT], BF16, tag="kT")
            qT = pers.tile([128, QT, NT], BF16, tag="qT")
            v_tok = pers.tile([128, NT // 128, HEAD_DIM], F32R, tag="vtok")

            # ---------------- phase 1: fused projections of hidden ----------
            with tc.tile_pool(name="p1c", bufs=1) as cp, \
                 tc.tile_pool(name="hid", bufs=4) as hp, \
                 tc.tile_pool(name="rps", bufs=2) as rpp, \
                 tc.tile_pool(name="st1", bufs=2) as st, \
                 tc.tile_pool(name="ps1", bufs=6, space=MS.PSUM) as pp, \
                 tc.tile_pool(name="psT", bufs=2, space=MS.PSUM) as pvt:
                ident_sb = cp.tile([128, 128], F32R, tag="id")
                nc.scalar.dma_start(ident_sb[:], identd.bitcast(F32R))
                wc_sb = cp.tile([128, HT, WC], BF16, tag="wc")
                wc_r = wcomb.rearrange("(t p) w -> p t w", p=128)
                for qtr in range(4):
                    t0, t1 = qtr * (HT // 4), (qtr + 1) * (HT // 4)
                    nc.scalar.dma_start(wc_sb[:, t0:t1, :], wc_r[:, t0:t1, :])

                for blk in range(NTB):
                    c0, c1 = blk * TB, (blk + 1) * TB
                    rp = rpp.tile([128, 4, TB], BF16, tag="rp")
                    nc.sync.dma_start(rp[:], ropes[:, :, c0:c1])
                    hts = []
                    for half in range(2):
                        ht = hp.tile([128, HT // 2, TB], BF16, tag="hid")
                        if blk == 0:
                            for q4 in range(2):
                                nc.sync.dma_start(
                                    ht[:, q4 * 8:(q4 + 1) * 8, :],
                                    hidT[half * 2048 + q4 * 1024:
                                         half * 2048 + (q4 + 1) * 1024, c0:c1]
                                    .rearrange("(t p) w -> p t w", p=128))
                        else:
                            nc.sync.dma_start(
                                ht[:],
                                hidT[half * 2048:(half + 1) * 2048, c0:c1]
                                .rearrange("(t p) w -> p t w", p=128))
                        hts.append(ht)
                    ps = [pp.tile([128, TB], f32, tag="ps1", name=f"ps{_m}")
                          for _m in range(6)]
                    for t in range(HT):
                        htt = hts[t // 16][:, t % 16, :]
                        for m in range(6):
                            nc.tensor.matmul(
                                ps[m][:],
                                wc_sb[:, t, m * 128:(m + 1) * 128],
                                htt,
                                start=(t == 0), stop=(t == HT - 1))
                    # ---- evict q tiles (rope via sign-folded tables) ----
                    for m in range(QT):
                        qraw = st.tile([128, TB], BF16, tag="qraw")
                        nc.scalar.copy(qraw[:], ps[m][:])
                        rot = st.tile([128, TB], BF16, tag="rot")
                        nc.sync.dma_start(rot[0:64, :], qraw[64:128, :])
                        nc.sync.dma_start(rot[64:128, :], qraw[0:64, :])
                        t1 = st.tile([128, TB], BF16, tag="t1")
                        nc.vector.tensor_mul(t1[:], qraw[:], rp[:, 0, :])
                        nc.vector.tensor_mul(rot[:], rot[:], rp[:, 1, :])
                        nc.vector.tensor_add(qT[:, m, c0:c1], t1[:], rot[:])
                    # ---- evict kv tile: rows 0:32 krA, 32:64 kcA,
                    #      64:96 krB, 96:128 kcB (host pre-permuted) ----
                    kk = st.tile([128, TB], BF16, tag="kk")
                    nc.scalar.copy(kk[:], ps[4][:])
                    rt = st.tile([128, TB], BF16, tag="rt")
                    nc.sync.dma_start(rt[0:32, :], kk[64:96, :])
                    nc.sync.dma_start(rt[64:96, :], kk[0:32, :])
                    t2 = st.tile([128, TB], BF16, tag="t2")
                    for r0 in (0, 64):
                        nc.vector.tensor_mul(
                            t2[r0:r0 + 32, :], kk[r0:r0 + 32, :],
                            rp[r0:r0 + 32, 2, :])
                        nc.vector.tensor_mul(
                            rt[r0:r0 + 32, :], rt[r0:r0 + 32, :],
                            rp[r0:r0 + 32, 3, :])
                        nc.vector.tensor_add(
                            kT[r0:r0 + 32, c0:c1], t2[r0:r0 + 32, :],
                            rt[r0:r0 + 32, :])
                    nc.scalar.copy(kT[32:64, c0:c1], kk[32:64, :])
                    nc.scalar.copy(kT[96:128, c0:c1], kk[96:128, :])
                    # ---- evict v tile: vT [d, tok] -> transpose to [tok, d] --
                    vst = st.tile([128, TB], F32R, tag="vst")
                    nc.scalar.copy(vst[:], ps[5][:])
                    for tt in range(TB // 128):
                        vtp = pvt.tile([128, 128], F32R, tag="vtp")
                        nc.tensor.transpose(
                            vtp[:], vst[:, tt * 128:(tt + 1) * 128], ident_sb[:])
                        nc.vector.tensor_copy(
                            v_tok[:, blk * (TB // 128) + tt, :], vtp[:])

            # ---------------- phases 3+4 interleaved ----------------
            with tc.tile_pool(name="wop", bufs=1) as wop, \
                 tc.tile_pool(name="attn", bufs=1) as ap_, \
                 tc.tile_pool(name="c3", bufs=1) as cst3, \
                 tc.tile_pool(name="pt", bufs=4) as ptp, \
                 tc.tile_pool(name="pf", bufs=3) as pfp, \
                 tc.tile_pool(name="sm", bufs=2) as smp, \
                 tc.tile_pool(name="st4", bufs=2) as st4, \
                 tc.tile_pool(name="psS", bufs=2, space=MS.PSUM) as psS, \
                 tc.tile_pool(name="psP", bufs=2, space=MS.PSUM) as psP, \
                 tc.tile_pool(name="psO", bufs=2, space=MS.PSUM) as psO, \
                 tc.tile_pool(name="psU", bufs=2, space=MS.PSUM) as psU:
                masks_sb = cst3.tile([128, NJ, QB], F32R, tag="masks")
                nc.scalar.dma_start(masks_sb[:], masks.bitcast(F32R))
                ones_sb = cst3.tile([128, 1], F32R, tag="ones")
                nc.scalar.dma_start(ones_sb[:], onesd.bitcast(F32R))
                wo_sb = wop.tile([128, QT, HIDDEN], BF16, tag="wo")
                nc.scalar.dma_start(wo_sb[:], wo.rearrange("(t p) w -> p t w", p=128))
                attn_sb = ap_.tile([128, QT, NT], BF16, tag="attn")

                def emit_attn(b, qb, filler):
                    off = b * Sv
                    q0 = off + qb * QB
                    nkt = NJ * (qb + 1)         # causal k tiles for this block
                    # spread the previous block's o_proj units evenly over
                    # all drain slots (after each k-tile's scores + at each
                    # h boundary, where the exp-latency bubbles sit)
                    nfill = 65
                    slots = QT * (nkt + 1)
                    base, rem = divmod(nfill, slots)
                    slot = [0]

                    def drain():
                        k = base + (1 if slot[0] < rem else 0)
                        slot[0] += 1
                        for _ in range(k):
                            next(filler, None)

                    for h in range(QT):
                        ops = psO.tile([128, QB], f32, tag="ops")
                        sps = psU.tile([1, QB], f32, tag="sps")
                        for kt in range(nkt):
                            j = kt - NJ * qb        # diag idx if >= 0
                            tr = j * 128 if j > 0 else 0  # trim cols
                            scp = psS.tile([128, QB], f32, tag="scp")
                            nc.tensor.matmul(
                                scp[:, tr:QB],
                                kT[:, off + kt * 128: off + (kt + 1) * 128],
                                qT[:, h, q0 + tr:q0 + QB],
                                start=True, stop=True)
                            # o_proj filler: fills the PE bubble while the
                            # activation engine computes exp of this tile
                            drain()
                            ptile = ptp.tile([128, QB], F32R, tag="pt")
                            nc.scalar.activation(
                                ptile[:, tr:QB], scp[:, tr:QB], EXP)
                            if j >= 0:   # diagonal tile: causal mask
                                nc.vector.tensor_mul(
                                    ptile[:, tr:QB], ptile[:, tr:QB],
                                    masks_sb[:, j, tr:QB])
                            first = (kt == 0)
                            last = (kt == nkt - 1)
                            nc.tensor.matmul(
                                ops[:, tr:QB],
                                v_tok[:, b * NKT_B + kt, :],
                                ptile[:, tr:QB],
                                start=first, stop=last)
                            nc.tensor.matmul(
                                sps[:, tr:QB],
                                ones_sb[:],
                                ptile[:, tr:QB],
                                start=first, stop=last)
                        rec = smp.tile([1, QB], f32, tag="rec")
                        nc.vector.reciprocal_approx_fast(rec[:], sps[:])
                        rb = smp.tile([128, QB], f32, tag="rb")
                        nc.gpsimd.partition_broadcast(rb[:], rec[:])
                        nc.vector.tensor_mul(
                            attn_sb[:, h, q0:q0 + QB], ops[:], rb[:])
                        drain()
                    # drain any leftover filler units
                    for _ in filler:
                        pass

                def oproj_units(b, qb):
                    """Generator: each next() emits one 4-matmul o_proj unit
                    (one 512-wide hidden chunk) for this token block."""
                    q0 = b * Sv + qb * QB
                    for Tt in range(QB // 128):
                        T0 = q0 + Tt * 128
                        orow = st4.tile([128, HIDDEN], BF16, tag="orow")
                        for n in range(HIDDEN // 512):
                            po = psP.tile([128, QB], f32, tag="po")
                            for h2 in range(QT):
                                nc.tensor.matmul(
                                    po[:],
                                    attn_sb[:, h2, T0:T0 + 128],
                                    wo_sb[:, h2, n * 512:(n + 1) * 512],
                                    start=(h2 == 0), stop=(h2 == QT - 1))
                            yield
                            if n % 4 == 3:
                                nc.scalar.copy(
                                    orow[:, n * 512:(n + 1) * 512], po[:])
                            else:
                                nc.vector.tensor_copy(
                                    orow[:, n * 512:(n + 1) * 512], po[:])
                        nc.sync.dma_start(outp[T0:T0 + 128, :], orow[:])
                    yield

                blocks = [(b, qb) for b in range(Bv)
                          for qb in range(NQB - 1, -1, -1)]
                prev = None
                for blk in blocks:
                    filler = oproj_units(*prev) if prev is not None else iter(())
                    emit_attn(*blk, filler)
                    prev = blk
                for _ in oproj_units(*prev):
                    pass

    nc.compile()
    return nc


def make_in_maps(hidden_states, Wq, Wkr, Wdk, Wupk, Wupv, Wo, Bv=B, Sv=S, QB=512):
    """Host-side sharding + layout prep. Returns per-core input dicts."""
    import ml_dtypes
    bf16 = ml_dtypes.bfloat16
    fp8 = ml_dtypes.float8_e4m3fn

    NT = Bv * Sv
    NJ = QB // 128
    scale = 1.0 / np.sqrt(np.float32(HEAD_DIM))

    hidden_states = np.asarray(hidden_states, dtype=np.float32)
    Wq = np.asarray(Wq, dtype=np.float32)
    Wkr = np.asarray(Wkr, dtype=np.float32)
    Wdk = np.asarray(Wdk, dtype=np.float32)
    Wupk = np.asarray(Wupk, dtype=np.float32)
    Wupv = np.asarray(Wupv, dtype=np.float32)
    Wo = np.asarray(Wo, dtype=np.float32)

    hidT = np.ascontiguousarray(
        hidden_states.reshape(NT, HIDDEN).T).astype(bf16)

    cos_t, sin_t = _rope_tables(Sv)                    # [128, S]
    cos_t = np.tile(cos_t, (1, Bv))                    # [128, NT]
    sin_t = np.tile(sin_t, (1, Bv))
    qcos = cos_t * scale
    qsin = np.concatenate([-sin_t[0:64], sin_t[64:128]], axis=0) * scale
    # k rope tables in the permuted kv-row layout:
    #   rows 0:32  = rotated rope dims 0:32   (krA):  cos[0:32],  -sin[0:32]
    #   rows 64:96 = rotated rope dims 64:96  (krB):  cos[64:96], +sin[64:96]
    kcos = np.zeros((128, NT), np.float32)
    ksin = np.zeros((128, NT), np.float32)
    kcos[0:32] = cos_t[0:32]
    kcos[64:96] = cos_t[64:96]
    ksin[0:32] = -sin_t[0:32]
    ksin[64:96] = sin_t[64:96]
    ropes = np.ascontiguousarray(
        np.stack([qcos, qsin, kcos, ksin], axis=1)).astype(bf16)  # [128,4,NT]

    k_idx = np.arange(128)[:, None]
    q_idx = np.arange(QB)[None, :]
    masks = np.stack(
        [(q_idx >= j * 128 + k_idx).astype(np.float32) for j in range(NJ)],
        axis=1)                                        # [128, NJ, QB]
    masks = np.ascontiguousarray(masks)

    in_maps = []
    for c in range(NCORES):
        wq_t = Wq[QR * c:QR * (c + 1)].T               # [HIDDEN, 512]
        wkr_c = Wkr[KRR * c:KRR * (c + 1)]             # [64, HIDDEN]
        wfk_c = Wupk[KRR * c:KRR * (c + 1)] @ Wdk      # [64, HIDDEN] fused
        wfv_c = Wupv[HEAD_DIM * c:HEAD_DIM * (c + 1)] @ Wdk  # [128, HIDDEN]
        kvrows = np.empty((128, HIDDEN), np.float32)
        kvrows[0:32] = wkr_c[0:32]     # krA: rope dims 0:32
        kvrows[32:64] = wfk_c[0:32]    # kcA: nope dims 32:64
        kvrows[64:96] = wkr_c[32:64]   # krB: rope dims 64:96
        kvrows[96:128] = wfk_c[32:64]  # kcB: nope dims 96:128
        wcomb = np.concatenate([wq_t, kvrows.T, wfv_c.T], axis=1)  # [HIDDEN, 768]
        wo_t = Wo[:, QR * c:QR * (c + 1)].T            # [512, HIDDEN]
        in_maps.append({
            "hidT": hidT,
            "wcomb": np.ascontiguousarray(wcomb).astype(bf16),
            "wo_t": np.ascontiguousarray(wo_t).astype(bf16),
            "ropes": ropes,
            "masks": masks,
            "ones": np.ones((128, 1), np.float32),
            "ident": np.eye(128, dtype=np.float32),
        })
    return in_maps


_NC_CACHE = {}


def _get_program(key=(B, S, 512, 512)):
    if key not in _NC_CACHE:
        _NC_CACHE[key] = build_program(*key)
    return _NC_CACHE[key]


def kernel(hidden_states, Wq, Wkr, Wdk, Wupk, Wupv, Wo):
    from concourse.bass_utils import run_bass_kernel_spmd

    in_maps = make_in_maps(np.asarray(hidden_states), np.asarray(Wq),
                           np.asarray(Wkr), np.asarray(Wdk), np.asarray(Wupk),
                           np.asarray(Wupv), np.asarray(Wo))
    nc = _get_program()
    res = run_bass_kernel_spmd(nc, in_maps, list(range(NCORES)))
    out = res.results[0]["out_part"].astype(np.float32)
    for i in range(1, NCORES):
        out = out + res.results[i]["out_part"].astype(np.float32)
    return out.reshape(B, S, HIDDEN).astype(np.float32)


# revision 16
# speedup vs baseline: 1.0420x; 1.0142x over previous
"""MLA (CustomLlamaMLAForInfer) Trainium2 Bass kernel, v2.

Sharding: tensor-parallel over heads across 8 NeuronCores. Core c owns
kv-head c and q-heads [4c, 4c+4). Every core sees the full token stream
(B*S = 4096 tokens); o_proj is computed against the core's 512
head-dims, producing a partial [4096, 4096] bf16 output that the host
sums across the 8 cores.

v2 changes vs baseline:
  - Host fuses Wupk/Wupv through Wdk (k_c = hid @ (Wupk_c Wdk).T etc.),
    removing the replicated 512-dim latent projection and its DRAM
    round trip entirely.
  - Single phase-1 pass over hidT: one 6-bank PSUM group per token
    block produces q (4 tiles), interleaved k_rope/k_nope (1 tile,
    weight columns pre-permuted so no cross-partition moves at evict),
    vT (1 tile, PE-transposed to [tok, d]).
  - bf16 operands on the PE except p/v (f32r), halving DMA traffic.
  - qT stays resident in SBUF (no DRAM round trip).
  - Attention: scores for 2 k-tiles accumulate into one 2-bank PSUM
    tile, one wide exp (N=1024) per group; softmax denominators via
    ones-matmul; reciprocal_approx_fast instead of iterative reciprocal.
  - o_proj interleaved per (b, qb) block right after its 4 heads
    finish, sharing PSUM banks with the scores pool; qb descending so
    the wo prefetch hides under the deepest attention block.
"""

import numpy as np

HIDDEN = 4096
N_HEADS = 32
KV_HEADS = 8
HEAD_DIM = 128
LOW_RANK = 64
TOP_K_ROPE = 32
ROPE_THETA = 10000.0
B, S = 2, 2048
NCORES = 8
HPC = N_HEADS // NCORES          # q heads per core = 4
QR = HPC * HEAD_DIM              # q rows per core = 512
CD = LOW_RANK * KV_HEADS         # latent dim = 512
KRR = 2 * TOP_K_ROPE             # rope rows per kv head = 64
WKV = 256                        # fused kv out rows: kr 64 + kc 64 + v 128
WC = QR + WKV                    # combined projection out rows = 768


def _rope_tables(seq_len):
    inv = 1.0 / (ROPE_THETA ** (np.arange(0, HEAD_DIM, 2, dtype=np.float32) / HEAD_DIM))
    pos = np.arange(seq_len, dtype=np.float32)
    fr = np.outer(pos, inv)
    emb = np.concatenate([fr, fr], axis=-1)          # [S, 128]
    return (np.cos(emb).T.astype(np.float32),        # [128, S]
            np.sin(emb).T.astype(np.float32))


def build_program(Bv=B, Sv=S, TB=512, QB=512, trace_sim=False):
    from concourse import bacc, tile, mybir
    import concourse.bass as bass

    f32 = mybir.dt.float32
    F32R = mybir.dt.float32r
    BF16 = mybir.dt.bfloat16
    FP8 = mybir.dt.float8e4
    DR = mybir.MatmulPerfMode.DoubleRow
    MS = bass.MemorySpace
    EXP = mybir.ActivationFunctionType.Exp

    NT = Bv * Sv                 # total tokens = 4096
    HT = HIDDEN // 128           # hidden tiles = 32
    NTB = NT // TB               # proj token blocks = 8
    NQB = Sv // QB               # q blocks per batch = 4
    NJ = QB // 128               # diagonal mask variants = 4
    NKT_B = Sv // 128            # k tiles per batch = 16
    QT = QR // 128               # q-head tiles per core = 4

    nc = bacc.Bacc("TRN2", target_bir_lowering=False, debug=False,
                   num_devices=NCORES)

    def din(name, shape, dt=BF16):
        return nc.dram_tensor(name, shape, dt, kind="ExternalInput").ap()

    hidT = din("hidT", [HIDDEN, NT])
    wcomb = din("wcomb", [HIDDEN, WC])
    wo = din("wo_t", [QR, HIDDEN])
    ropes = din("ropes", [128, 4, NT])   # 0=qcos 1=qsin 2=kcos 3=ksin
    masks = din("masks", [128, NJ, QB])
    onesd = din("ones", [128, 1])
    identd = din("ident", [128, 128])
    outp = nc.dram_tensor("out_part", [NT, HIDDEN], BF16, kind="ExternalOutput").ap()

    with tile.TileContext(nc, trace_sim=trace_sim) as tc:
        with tc.tile_pool(name="persist", bufs=1) as pers:
            kT = pers.tile([128, NT], BF16, tag="kT")
            qT = pers.tile([128, QT, NT], BF16, tag="qT")
            v_tok = pers.tile([128, NT // 128, HEAD_DIM], BF16, tag="vtok")

            # ---------------- phase 1: fused projections of hidden ----------
            with tc.tile_pool(name="p1c", bufs=1) as cp, \
                 tc.tile_pool(name="hid", bufs=4) as hp, \
                 tc.tile_pool(name="rps", bufs=2) as rpp, \
                 tc.tile_pool(name="st1", bufs=2) as st, \
                 tc.tile_pool(name="ps1", bufs=6, space=MS.PSUM) as pp, \
                 tc.tile_pool(name="psT", bufs=2, space=MS.PSUM) as pvt:
                ident_sb = cp.tile([128, 128], BF16, tag="id")
                nc.scalar.dma_start(ident_sb[:], identd)
                # pre-load the exp ACT table so the first attention tile
                # doesn't pay the ~1.5us table switch
                warm = cp.tile([1, 2], f32, tag="warm")
                nc.vector.memset(warm[0:1, 0:1], 0.0)
                nc.scalar.activation(warm[0:1, 1:2], warm[0:1, 0:1], EXP)
                wc_sb = cp.tile([128, HT, WC], BF16, tag="wc")
                wc_r = wcomb.rearrange("(t p) w -> p t w", p=128)
                for qtr in range(4):
                    t0, t1 = qtr * (HT // 4), (qtr + 1) * (HT // 4)
                    nc.scalar.dma_start(wc_sb[:, t0:t1, :], wc_r[:, t0:t1, :])

                for blk in range(NTB):
                    c0, c1 = blk * TB, (blk + 1) * TB
                    rp = rpp.tile([128, 4, TB], BF16, tag="rp")
                    nc.sync.dma_start(rp[:], ropes[:, :, c0:c1])
                    hts = []
                    for half in range(2):
                        ht = hp.tile([128, HT // 2, TB], BF16, tag="hid")
                        if blk == 0:
                            for q4 in range(2):
                                nc.sync.dma_start(
                                    ht[:, q4 * 8:(q4 + 1) * 8, :],
                                    hidT[half * 2048 + q4 * 1024:
                                         half * 2048 + (q4 + 1) * 1024, c0:c1]
                                    .rearrange("(t p) w -> p t w", p=128))
                        else:
                            nc.sync.dma_start(
                                ht[:],
                                hidT[half * 2048:(half + 1) * 2048, c0:c1]
                                .rearrange("(t p) w -> p t w", p=128))
                        hts.append(ht)
                    ps = [pp.tile([128, TB], f32, tag="ps1", name=f"ps{_m}")
                          for _m in range(6)]
                    for t in range(HT):
                        htt = hts[t // 16][:, t % 16, :]
                        for m in range(6):
                            nc.tensor.matmul(
                                ps[m][:],
                                wc_sb[:, t, m * 128:(m + 1) * 128],
                                htt,
                                start=(t == 0), stop=(t == HT - 1))
                    # ---- evict q tiles (rope via sign-folded tables) ----
                    for m in range(QT):
                        qraw = st.tile([128, TB], BF16, tag="qraw")
                        nc.scalar.copy(qraw[:], ps[m][:])
                        rot = st.tile([128, TB], BF16, tag="rot")
                        nc.sync.dma_start(rot[0:64, :], qraw[64:128, :])
                        nc.sync.dma_start(rot[64:128, :], qraw[0:64, :])
                        t1 = st.tile([128, TB], BF16, tag="t1")
                        nc.vector.tensor_mul(t1[:], qraw[:], rp[:, 0, :])
                        nc.vector.tensor_mul(rot[:], rot[:], rp[:, 1, :])
                        nc.vector.tensor_add(qT[:, m, c0:c1], t1[:], rot[:])
                    # ---- evict kv tile: rows 0:32 krA, 32:64 kcA,
                    #      64:96 krB, 96:128 kcB (host pre-permuted) ----
                    kk = st.tile([128, TB], BF16, tag="kk")
                    nc.scalar.copy(kk[:], ps[4][:])
                    rt = st.tile([128, TB], BF16, tag="rt")
                    nc.sync.dma_start(rt[0:32, :], kk[64:96, :])
                    nc.sync.dma_start(rt[64:96, :], kk[0:32, :])
                    t2 = st.tile([128, TB], BF16, tag="t2")
                    for r0 in (0, 64):
                        nc.vector.tensor_mul(
                            t2[r0:r0 + 32, :], kk[r0:r0 + 32, :],
                            rp[r0:r0 + 32, 2, :])
                        nc.vector.tensor_mul(
                            rt[r0:r0 + 32, :], rt[r0:r0 + 32, :],
                            rp[r0:r0 + 32, 3, :])
                        nc.vector.tensor_add(
                            kT[r0:r0 + 32, c0:c1], t2[r0:r0 + 32, :],
                            rt[r0:r0 + 32, :])
                    nc.scalar.copy(kT[32:64, c0:c1], kk[32:64, :])
                    nc.scalar.copy(kT[96:128, c0:c1], kk[96:128, :])
                    # ---- evict v tile: vT [d, tok] -> transpose to [tok, d] --
                    vst = st.tile([128, TB], BF16, tag="vst")
                    nc.scalar.copy(vst[:], ps[5][:])
                    for tt in range(TB // 128):
                        vtp = pvt.tile([128, 128], BF16, tag="vtp")
                        nc.tensor.transpose(
                            vtp[:], vst[:, tt * 128:(tt + 1) * 128], ident_sb[:])
                        nc.vector.tensor_copy(
                            v_tok[:, blk * (TB // 128) + tt, :], vtp[:])

            # ---------------- phases 3+4 interleaved ----------------
            with tc.tile_pool(name="wop", bufs=1) as wop, \
                 tc.tile_pool(name="attn", bufs=1) as ap_, \
                 tc.tile_pool(name="c3", bufs=1) as cst3, \
                 tc.tile_pool(name="pt", bufs=4) as ptp, \
                 tc.tile_pool(name="pf", bufs=3) as pfp, \
                 tc.tile_pool(name="sm", bufs=2) as smp, \
                 tc.tile_pool(name="st4", bufs=2) as st4, \
                 tc.tile_pool(name="psS", bufs=3, space=MS.PSUM) as psS, \
                 tc.tile_pool(name="psP", bufs=2, space=MS.PSUM) as psP, \
                 tc.tile_pool(name="psO", bufs=2, space=MS.PSUM) as psO, \
                 tc.tile_pool(name="psU", bufs=1, space=MS.PSUM) as psU:
                masks_sb = cst3.tile([128, NJ, QB], BF16, tag="masks")
                nc.scalar.dma_start(masks_sb[:], masks)
                ones_sb = cst3.tile([128, 1], BF16, tag="ones")
                nc.scalar.dma_start(ones_sb[:], onesd)
                wo_sb = wop.tile([128, QT, HIDDEN], BF16, tag="wo")
                nc.scalar.dma_start(wo_sb[:], wo.rearrange("(t p) w -> p t w", p=128))
                attn_sb = ap_.tile([128, QT, NT], BF16, tag="attn")

                def emit_attn(b, qb, filler):
                    off = b * Sv
                    q0 = off + qb * QB
                    nkt = NJ * (qb + 1)         # causal k tiles for this block
                    # spread the previous block's o_proj units evenly over
                    # all drain slots (after each k-tile's scores + at each
                    # h boundary, where the exp-latency bubbles sit)
                    nfill = 65
                    slots = QT * (nkt + 1)
                    base, rem = divmod(nfill, slots)
                    slot = [0]

                    def drain():
                        k = base + (1 if slot[0] < rem else 0)
                        slot[0] += 1
                        for _ in range(k):
                            next(filler, None)

                    for h in range(QT):
                        ops = psO.tile([128, QB], f32, tag="ops")
                        sps = psU.tile([1, QB], f32, tag="sps")
                        for kt in range(nkt):
                            j = kt - NJ * qb        # diag idx if >= 0
                            tr = j * 128 if j > 0 else 0  # trim cols
                            scp = psS.tile([128, QB], f32, tag="scp")
                            nc.tensor.matmul(
                                scp[:, tr:QB],
                                kT[:, off + kt * 128: off + (kt + 1) * 128],
                                qT[:, h, q0 + tr:q0 + QB],
                                start=True, stop=True)
                            # o_proj filler: fills the PE bubble while the
                            # activation engine computes exp of this tile
                            drain()
                            ptile = ptp.tile([128, QB], BF16, tag="pt")
                            nc.scalar.activation(
                                ptile[:, tr:QB], scp[:, tr:QB], EXP)
                            if j >= 0:   # diagonal tile: causal mask
                                nc.vector.tensor_mul(
                                    ptile[:, tr:QB], ptile[:, tr:QB],
                                    masks_sb[:, j, tr:QB])
                            first = (kt == 0)
                            last = (kt == nkt - 1)
                            nc.tensor.matmul(
                                ops[:, tr:QB],
                                v_tok[:, b * NKT_B + kt, :],
                                ptile[:, tr:QB],
                                start=first, stop=last)
                            nc.tensor.matmul(
                                sps[:, tr:QB],
                                ones_sb[:],
                                ptile[:, tr:QB],
                                start=first, stop=last)
                        rec = smp.tile([1, QB], f32, tag="rec")
                        nc.vector.reciprocal_approx_fast(rec[:], sps[:])
                        rb = smp.tile([128, QB], f32, tag="rb")
                        nc.gpsimd.partition_broadcast(rb[:], rec[:])
                        nc.vector.tensor_mul(
                            attn_sb[:, h, q0:q0 + QB], ops[:], rb[:])
                        drain()
                    # drain any leftover filler units
                    for _ in filler:
                        pass

                def oproj_units(b, qb):
                    """Generator: each next() emits one 4-matmul o_proj unit
                    (one 512-wide hidden chunk) for this token block."""
                    q0 = b * Sv + qb * QB
                    for Tt in range(QB // 128):
                        T0 = q0 + Tt * 128
                        orow = st4.tile([128, HIDDEN], BF16, tag="orow")
                        for n in range(HIDDEN // 512):
                            po = psP.tile([128, QB], f32, tag="po")
                            for h2 in range(QT):
                                nc.tensor.matmul(
                                    po[:],
                                    attn_sb[:, h2, T0:T0 + 128],
                                    wo_sb[:, h2, n * 512:(n + 1) * 512],
                                    start=(h2 == 0), stop=(h2 == QT - 1))
                            yield
                            if n % 4 == 3:
                                nc.scalar.copy(
                                    orow[:, n * 512:(n + 1) * 512], po[:])
                            else:
                                nc.vector.tensor_copy(
                                    orow[:, n * 512:(n + 1) * 512], po[:])
                            if n == 3:
                                nc.sync.dma_start(
                                    outp[T0:T0 + 128, 0:2048], orow[:, 0:2048])
                        nc.sync.dma_start(
                            outp[T0:T0 + 128, 2048:HIDDEN], orow[:, 2048:HIDDEN])
                    yield

                blocks = [(b, qb) for b in range(Bv)
                          for qb in range(NQB - 1, -1, -1)]
                prev = None
                for blk in blocks:
                    filler = oproj_units(*prev) if prev is not None else iter(())
                    emit_attn(*blk, filler)
                    prev = blk
                for _ in oproj_units(*prev):
                    pass

    nc.compile()
    return nc


def make_in_maps(hidden_states, Wq, Wkr, Wdk, Wupk, Wupv, Wo, Bv=B, Sv=S, QB=512):
    """Host-side sharding + layout prep. Returns per-core input dicts."""
    import ml_dtypes
    bf16 = ml_dtypes.bfloat16
    fp8 = ml_dtypes.float8_e4m3fn

    NT = Bv * Sv
    NJ = QB // 128
    scale = 1.0 / np.sqrt(np.float32(HEAD_DIM))

    hidden_states = np.asarray(hidden_states, dtype=np.float32)
    Wq = np.asarray(Wq, dtype=np.float32)
    Wkr = np.asarray(Wkr, dtype=np.float32)
    Wdk = np.asarray(Wdk, dtype=np.float32)
    Wupk = np.asarray(Wupk, dtype=np.float32)
    Wupv = np.asarray(Wupv, dtype=np.float32)
    Wo = np.asarray(Wo, dtype=np.float32)

    hidT = np.ascontiguousarray(
        hidden_states.reshape(NT, HIDDEN).T).astype(bf16)

    cos_t, sin_t = _rope_tables(Sv)                    # [128, S]
    cos_t = np.tile(cos_t, (1, Bv))                    # [128, NT]
    sin_t = np.tile(sin_t, (1, Bv))
    qcos = cos_t * scale
    qsin = np.concatenate([-sin_t[0:64], sin_t[64:128]], axis=0) * scale
    # k rope tables in the permuted kv-row layout:
    #   rows 0:32  = rotated rope dims 0:32   (krA):  cos[0:32],  -sin[0:32]
    #   rows 64:96 = rotated rope dims 64:96  (krB):  cos[64:96], +sin[64:96]
    kcos = np.zeros((128, NT), np.float32)
    ksin = np.zeros((128, NT), np.float32)
    kcos[0:32] = cos_t[0:32]
    kcos[64:96] = cos_t[64:96]
    ksin[0:32] = -sin_t[0:32]
    ksin[64:96] = sin_t[64:96]
    ropes = np.ascontiguousarray(
        np.stack([qcos, qsin, kcos, ksin], axis=1)).astype(bf16)  # [128,4,NT]

    k_idx = np.arange(128)[:, None]
    q_idx = np.arange(QB)[None, :]
    masks = np.stack(
        [(q_idx >= j * 128 + k_idx).astype(np.float32) for j in range(NJ)],
        axis=1)                                        # [128, NJ, QB]
    masks = np.ascontiguousarray(masks)

    in_maps = []
    for c in range(NCORES):
        wq_t = Wq[QR * c:QR * (c + 1)].T               # [HIDDEN, 512]
        wkr_c = Wkr[KRR * c:KRR * (c + 1)]             # [64, HIDDEN]
        wfk_c = Wupk[KRR * c:KRR * (c + 1)] @ Wdk      # [64, HIDDEN] fused
        wfv_c = Wupv[HEAD_DIM * c:HEAD_DIM * (c + 1)] @ Wdk  # [128, HIDDEN]
        kvrows = np.empty((128, HIDDEN), np.float32)
        kvrows[0:32] = wkr_c[0:32]     # krA: rope dims 0:32
        kvrows[32:64] = wfk_c[0:32]    # kcA: nope dims 32:64
        kvrows[64:96] = wkr_c[32:64]   # krB: rope dims 64:96
        kvrows[96:128] = wfk_c[32:64]  # kcB: nope dims 96:128
        wcomb = np.concatenate([wq_t, kvrows.T, wfv_c.T], axis=1)  # [HIDDEN, 768]
        wo_t = Wo[:, QR * c:QR * (c + 1)].T            # [512, HIDDEN]
        in_maps.append({
            "hidT": hidT,
            "wcomb": np.ascontiguousarray(wcomb).astype(bf16),
            "wo_t": np.ascontiguousarray(wo_t).astype(bf16),
            "ropes": ropes,
            "masks": masks.astype(bf16),
            "ones": np.ones((128, 1), bf16),
            "ident": np.eye(128, dtype=np.float32).astype(bf16),
        })
    return in_maps


_NC_CACHE = {}


def _get_program(key=(B, S, 512, 512)):
    if key not in _NC_CACHE:
        _NC_CACHE[key] = build_program(*key)
    return _NC_CACHE[key]


def kernel(hidden_states, Wq, Wkr, Wdk, Wupk, Wupv, Wo):
    from concourse.bass_utils import run_bass_kernel_spmd

    in_maps = make_in_maps(np.asarray(hidden_states), np.asarray(Wq),
                           np.asarray(Wkr), np.asarray(Wdk), np.asarray(Wupk),
                           np.asarray(Wupv), np.asarray(Wo))
    nc = _get_program()
    res = run_bass_kernel_spmd(nc, in_maps, list(range(NCORES)))
    out = res.results[0]["out_part"].astype(np.float32)
    for i in range(1, NCORES):
        out = out + res.results[i]["out_part"].astype(np.float32)
    return out.reshape(B, S, HIDDEN).astype(np.float32)


# revision 19
# speedup vs baseline: 1.0422x; 1.0002x over previous
"""MLA (CustomLlamaMLAForInfer) Trainium2 Bass kernel, v2.

Sharding: tensor-parallel over heads across 8 NeuronCores. Core c owns
kv-head c and q-heads [4c, 4c+4). Every core sees the full token stream
(B*S = 4096 tokens); o_proj is computed against the core's 512
head-dims, producing a partial [4096, 4096] bf16 output that the host
sums across the 8 cores.

v2 changes vs baseline:
  - Host fuses Wupk/Wupv through Wdk (k_c = hid @ (Wupk_c Wdk).T etc.),
    removing the replicated 512-dim latent projection and its DRAM
    round trip entirely.
  - Single phase-1 pass over hidT: one 6-bank PSUM group per token
    block produces q (4 tiles), interleaved k_rope/k_nope (1 tile,
    weight columns pre-permuted so no cross-partition moves at evict),
    vT (1 tile, PE-transposed to [tok, d]).
  - bf16 operands on the PE except p/v (f32r), halving DMA traffic.
  - qT stays resident in SBUF (no DRAM round trip).
  - Attention: scores for 2 k-tiles accumulate into one 2-bank PSUM
    tile, one wide exp (N=1024) per group; softmax denominators via
    ones-matmul; reciprocal_approx_fast instead of iterative reciprocal.
  - o_proj interleaved per (b, qb) block right after its 4 heads
    finish, sharing PSUM banks with the scores pool; qb descending so
    the wo prefetch hides under the deepest attention block.
"""

import numpy as np

HIDDEN = 4096
N_HEADS = 32
KV_HEADS = 8
HEAD_DIM = 128
LOW_RANK = 64
TOP_K_ROPE = 32
ROPE_THETA = 10000.0
B, S = 2, 2048
NCORES = 8
HPC = N_HEADS // NCORES          # q heads per core = 4
QR = HPC * HEAD_DIM              # q rows per core = 512
CD = LOW_RANK * KV_HEADS         # latent dim = 512
KRR = 2 * TOP_K_ROPE             # rope rows per kv head = 64
WKV = 256                        # fused kv out rows: kr 64 + kc 64 + v 128
WC = QR + WKV                    # combined projection out rows = 768


def _rope_tables(seq_len):
    inv = 1.0 / (ROPE_THETA ** (np.arange(0, HEAD_DIM, 2, dtype=np.float32) / HEAD_DIM))
    pos = np.arange(seq_len, dtype=np.float32)
    fr = np.outer(pos, inv)
    emb = np.concatenate([fr, fr], axis=-1)          # [S, 128]
    return (np.cos(emb).T.astype(np.float32),        # [128, S]
            np.sin(emb).T.astype(np.float32))


def build_program(Bv=B, Sv=S, TB=512, QB=512, trace_sim=False):
    from concourse import bacc, tile, mybir
    import concourse.bass as bass

    f32 = mybir.dt.float32
    F32R = mybir.dt.float32r
    BF16 = mybir.dt.bfloat16
    FP8 = mybir.dt.float8e4
    DR = mybir.MatmulPerfMode.DoubleRow
    MS = bass.MemorySpace
    EXP = mybir.ActivationFunctionType.Exp

    NT = Bv * Sv                 # total tokens = 4096
    HT = HIDDEN // 128           # hidden tiles = 32
    NTB = NT // TB               # proj token blocks = 8
    NQB = Sv // QB               # q blocks per batch = 4
    NJ = QB // 128               # diagonal mask variants = 4
    NKT_B = Sv // 128            # k tiles per batch = 16
    QT = QR // 128               # q-head tiles per core = 4

    nc = bacc.Bacc("TRN2", target_bir_lowering=False, debug=False,
                   num_devices=NCORES)

    def din(name, shape, dt=BF16):
        return nc.dram_tensor(name, shape, dt, kind="ExternalInput").ap()

    hidT = din("hidT", [HIDDEN, NT])
    wcomb = din("wcomb", [HIDDEN, WC])
    wo = din("wo_t", [QR, HIDDEN])
    ropes = din("ropes", [128, 4, NT])   # 0=qcos 1=qsin 2=kcos 3=ksin
    masks = din("masks", [128, NJ, QB])
    onesd = din("ones", [128, 1])
    identd = din("ident", [128, 128])
    outp = nc.dram_tensor("out_part", [NT, HIDDEN], BF16, kind="ExternalOutput").ap()

    with tile.TileContext(nc, trace_sim=trace_sim) as tc:
        with tc.tile_pool(name="persist", bufs=1) as pers:
            kT = pers.tile([128, NT], BF16, tag="kT")
            qT = pers.tile([128, QT, NT], BF16, tag="qT")
            v_tok = pers.tile([128, NT // 128, HEAD_DIM], BF16, tag="vtok")

            # phase-3 constants + weights live in the persistent pool so
            # their DMAs can queue on the scalar ring ahead of phase-1's
            # ACT eviction work
            masks_sb = pers.tile([128, NJ, QB], BF16, tag="masks")
            ones_sb = pers.tile([128, 1], BF16, tag="ones")
            wo_sb = pers.tile([128, QT, HIDDEN], BF16, tag="wo")

            # ---------------- phase 1: fused projections of hidden ----------
            with tc.tile_pool(name="p1c", bufs=1) as cp, \
                 tc.tile_pool(name="hid", bufs=3) as hp, \
                 tc.tile_pool(name="rps", bufs=2) as rpp, \
                 tc.tile_pool(name="st1", bufs=2) as st, \
                 tc.tile_pool(name="ps1", bufs=6, space=MS.PSUM) as pp, \
                 tc.tile_pool(name="psT", bufs=2, space=MS.PSUM) as pvt:
                ident_sb = cp.tile([128, 128], BF16, tag="id")
                nc.scalar.dma_start(ident_sb[:], identd)
                nc.scalar.dma_start(masks_sb[:], masks)
                nc.scalar.dma_start(ones_sb[:], onesd)
                # pre-load the exp ACT table so the first attention tile
                # doesn't pay the ~1.5us table switch
                warm = cp.tile([1, 2], f32, tag="warm")
                nc.vector.memset(warm[0:1, 0:1], 0.0)
                nc.scalar.activation(warm[0:1, 1:2], warm[0:1, 0:1], EXP)
                wc_sb = cp.tile([128, HT, WC], BF16, tag="wc")
                wc_r = wcomb.rearrange("(t p) w -> p t w", p=128)
                for qtr in range(4):
                    t0, t1 = qtr * (HT // 4), (qtr + 1) * (HT // 4)
                    nc.scalar.dma_start(wc_sb[:, t0:t1, :], wc_r[:, t0:t1, :])
                nc.scalar.dma_start(
                    wo_sb[:], wo.rearrange("(t p) w -> p t w", p=128))

                for blk in range(NTB):
                    c0, c1 = blk * TB, (blk + 1) * TB
                    rp = rpp.tile([128, 4, TB], BF16, tag="rp")
                    nc.sync.dma_start(rp[:], ropes[:, :, c0:c1])
                    hts = []
                    for half in range(2):
                        ht = hp.tile([128, HT // 2, TB], BF16, tag="hid")
                        if blk == 0:
                            for q4 in range(2):
                                nc.sync.dma_start(
                                    ht[:, q4 * 8:(q4 + 1) * 8, :],
                                    hidT[half * 2048 + q4 * 1024:
                                         half * 2048 + (q4 + 1) * 1024, c0:c1]
                                    .rearrange("(t p) w -> p t w", p=128))
                        else:
                            nc.sync.dma_start(
                                ht[:],
                                hidT[half * 2048:(half + 1) * 2048, c0:c1]
                                .rearrange("(t p) w -> p t w", p=128))
                        hts.append(ht)
                    ps = [pp.tile([128, TB], f32, tag="ps1", name=f"ps{_m}")
                          for _m in range(6)]
                    for t in range(HT):
                        htt = hts[t // 16][:, t % 16, :]
                        for m in range(6):
                            nc.tensor.matmul(
                                ps[m][:],
                                wc_sb[:, t, m * 128:(m + 1) * 128],
                                htt,
                                start=(t == 0), stop=(t == HT - 1))
                    # ---- evict q tiles (rope via sign-folded tables) ----
                    for m in range(QT):
                        qraw = st.tile([128, TB], BF16, tag="qraw")
                        nc.scalar.copy(qraw[:], ps[m][:])
                        rot = st.tile([128, TB], BF16, tag="rot")
                        nc.sync.dma_start(rot[0:64, :], qraw[64:128, :])
                        nc.sync.dma_start(rot[64:128, :], qraw[0:64, :])
                        t1 = st.tile([128, TB], BF16, tag="t1")
                        nc.vector.tensor_mul(t1[:], qraw[:], rp[:, 0, :])
                        nc.vector.tensor_mul(rot[:], rot[:], rp[:, 1, :])
                        nc.vector.tensor_add(qT[:, m, c0:c1], t1[:], rot[:])
                    # ---- evict kv tile: rows 0:32 krA, 32:64 kcA,
                    #      64:96 krB, 96:128 kcB (host pre-permuted) ----
                    kk = st.tile([128, TB], BF16, tag="kk")
                    nc.scalar.copy(kk[:], ps[4][:])
                    rt = st.tile([128, TB], BF16, tag="rt")
                    nc.sync.dma_start(rt[0:32, :], kk[64:96, :])
                    nc.sync.dma_start(rt[64:96, :], kk[0:32, :])
                    t2 = st.tile([128, TB], BF16, tag="t2")
                    for r0 in (0, 64):
                        nc.vector.tensor_mul(
                            t2[r0:r0 + 32, :], kk[r0:r0 + 32, :],
                            rp[r0:r0 + 32, 2, :])
                        nc.vector.tensor_mul(
                            rt[r0:r0 + 32, :], rt[r0:r0 + 32, :],
                            rp[r0:r0 + 32, 3, :])
                        nc.vector.tensor_add(
                            kT[r0:r0 + 32, c0:c1], t2[r0:r0 + 32, :],
                            rt[r0:r0 + 32, :])
                    nc.scalar.copy(kT[32:64, c0:c1], kk[32:64, :])
                    nc.scalar.copy(kT[96:128, c0:c1], kk[96:128, :])
                    # ---- evict v tile: vT [d, tok] -> transpose to [tok, d] --
                    vst = st.tile([128, TB], BF16, tag="vst")
                    nc.scalar.copy(vst[:], ps[5][:])
                    for tt in range(TB // 128):
                        vtp = pvt.tile([128, 128], BF16, tag="vtp")
                        nc.tensor.transpose(
                            vtp[:], vst[:, tt * 128:(tt + 1) * 128], ident_sb[:])
                        nc.vector.tensor_copy(
                            v_tok[:, blk * (TB // 128) + tt, :], vtp[:])

            # ---------------- phases 3+4 interleaved ----------------
            with tc.tile_pool(name="attn", bufs=1) as ap_, \
                 tc.tile_pool(name="pt", bufs=4) as ptp, \
                 tc.tile_pool(name="pf", bufs=3) as pfp, \
                 tc.tile_pool(name="sm", bufs=2) as smp, \
                 tc.tile_pool(name="st4", bufs=2) as st4, \
                 tc.tile_pool(name="psS", bufs=3, space=MS.PSUM) as psS, \
                 tc.tile_pool(name="psP", bufs=2, space=MS.PSUM) as psP, \
                 tc.tile_pool(name="psO", bufs=2, space=MS.PSUM) as psO, \
                 tc.tile_pool(name="psU", bufs=1, space=MS.PSUM) as psU:
                attn_sb = ap_.tile([128, QT, NT], BF16, tag="attn")

                def emit_attn(b, qb, filler):
                    off = b * Sv
                    q0 = off + qb * QB
                    nkt = NJ * (qb + 1)         # causal k tiles for this block
                    # spread the previous block's o_proj units evenly over
                    # all drain slots (after each k-tile's scores + at each
                    # h boundary, where the exp-latency bubbles sit)
                    nfill = 65
                    slots = QT * (nkt + 1)
                    base, rem = divmod(nfill, slots)
                    slot = [0]

                    def drain():
                        k = base + (1 if slot[0] < rem else 0)
                        slot[0] += 1
                        for _ in range(k):
                            next(filler, None)

                    for h in range(QT):
                        ops = psO.tile([128, QB], f32, tag="ops")
                        sps = psU.tile([1, QB], f32, tag="sps")
                        for kt in range(nkt):
                            j = kt - NJ * qb        # diag idx if >= 0
                            tr = j * 128 if j > 0 else 0  # trim cols
                            scp = psS.tile([128, QB], f32, tag="scp")
                            nc.tensor.matmul(
                                scp[:, tr:QB],
                                kT[:, off + kt * 128: off + (kt + 1) * 128],
                                qT[:, h, q0 + tr:q0 + QB],
                                start=True, stop=True)
                            # o_proj filler: fills the PE bubble while the
                            # activation engine computes exp of this tile
                            drain()
                            ptile = ptp.tile([128, QB], BF16, tag="pt")
                            nc.scalar.activation(
                                ptile[:, tr:QB], scp[:, tr:QB], EXP)
                            if j >= 0:   # diagonal tile: causal mask
                                nc.vector.tensor_mul(
                                    ptile[:, tr:QB], ptile[:, tr:QB],
                                    masks_sb[:, j, tr:QB])
                            first = (kt == 0)
                            last = (kt == nkt - 1)
                            nc.tensor.matmul(
                                ops[:, tr:QB],
                                v_tok[:, b * NKT_B + kt, :],
                                ptile[:, tr:QB],
                                start=first, stop=last)
                            nc.tensor.matmul(
                                sps[:, tr:QB],
                                ones_sb[:],
                                ptile[:, tr:QB],
                                start=first, stop=last)
                        rec = smp.tile([1, QB], f32, tag="rec")
                        nc.vector.reciprocal_approx_fast(rec[:], sps[:])
                        rb = smp.tile([128, QB], f32, tag="rb")
                        nc.gpsimd.partition_broadcast(rb[:], rec[:])
                        nc.vector.tensor_mul(
                            attn_sb[:, h, q0:q0 + QB], ops[:], rb[:])
                        drain()
                    # drain any leftover filler units
                    for _ in filler:
                        pass

                def oproj_units(b, qb):
                    """Generator: each next() emits one 4-matmul o_proj unit
                    (one 1024-wide bf16-psum hidden chunk) for this block."""
                    q0 = b * Sv + qb * QB
                    for Tt in range(QB // 128):
                        T0 = q0 + Tt * 128
                        orow = st4.tile([128, HIDDEN], BF16, tag="orow")
                        for n in range(HIDDEN // 512):
                            po = psP.tile([128, QB], f32, tag="po")
                            for h2 in range(QT):
                                nc.tensor.matmul(
                                    po[:],
                                    attn_sb[:, h2, T0:T0 + 128],
                                    wo_sb[:, h2, n * 512:(n + 1) * 512],
                                    start=(h2 == 0), stop=(h2 == QT - 1))
                            yield
                            if n % 4 == 3:
                                nc.scalar.copy(
                                    orow[:, n * 512:(n + 1) * 512], po[:])
                            else:
                                nc.vector.tensor_copy(
                                    orow[:, n * 512:(n + 1) * 512], po[:])
                            if n % 2 == 1:
                                nc.sync.dma_start(
                                    outp[T0:T0 + 128, (n - 1) * 512:(n + 1) * 512],
                                    orow[:, (n - 1) * 512:(n + 1) * 512])
                    yield

                blocks = [(b, qb) for b in range(Bv)
                          for qb in range(NQB - 1, -1, -1)]
                prev = None
                for blk in blocks:
                    filler = oproj_units(*prev) if prev is not None else iter(())
                    emit_attn(*blk, filler)
                    prev = blk
                for _ in oproj_units(*prev):
                    pass

    nc.compile()
    return nc


def make_in_maps(hidden_states, Wq, Wkr, Wdk, Wupk, Wupv, Wo, Bv=B, Sv=S, QB=512):
    """Host-side sharding + layout prep. Returns per-core input dicts."""
    import ml_dtypes
    bf16 = ml_dtypes.bfloat16
    fp8 = ml_dtypes.float8_e4m3fn

    NT = Bv * Sv
    NJ = QB // 128
    scale = 1.0 / np.sqrt(np.float32(HEAD_DIM))

    hidden_states = np.asarray(hidden_states, dtype=np.float32)
    Wq = np.asarray(Wq, dtype=np.float32)
    Wkr = np.asarray(Wkr, dtype=np.float32)
    Wdk = np.asarray(Wdk, dtype=np.float32)
    Wupk = np.asarray(Wupk, dtype=np.float32)
    Wupv = np.asarray(Wupv, dtype=np.float32)
    Wo = np.asarray(Wo, dtype=np.float32)

    hidT = np.ascontiguousarray(
        hidden_states.reshape(NT, HIDDEN).T).astype(bf16)

    cos_t, sin_t = _rope_tables(Sv)                    # [128, S]
    cos_t = np.tile(cos_t, (1, Bv))                    # [128, NT]
    sin_t = np.tile(sin_t, (1, Bv))
    qcos = cos_t * scale
    qsin = np.concatenate([-sin_t[0:64], sin_t[64:128]], axis=0) * scale
    # k rope tables in the permuted kv-row layout:
    #   rows 0:32  = rotated rope dims 0:32   (krA):  cos[0:32],  -sin[0:32]
    #   rows 64:96 = rotated rope dims 64:96  (krB):  cos[64:96], +sin[64:96]
    kcos = np.zeros((128, NT), np.float32)
    ksin = np.zeros((128, NT), np.float32)
    kcos[0:32] = cos_t[0:32]
    kcos[64:96] = cos_t[64:96]
    ksin[0:32] = -sin_t[0:32]
    ksin[64:96] = sin_t[64:96]
    ropes = np.ascontiguousarray(
        np.stack([qcos, qsin, kcos, ksin], axis=1)).astype(bf16)  # [128,4,NT]

    k_idx = np.arange(128)[:, None]
    q_idx = np.arange(QB)[None, :]
    masks = np.stack(
        [(q_idx >= j * 128 + k_idx).astype(np.float32) for j in range(NJ)],
        axis=1)                                        # [128, NJ, QB]
    masks = np.ascontiguousarray(masks)

    in_maps = []
    for c in range(NCORES):
        wq_t = Wq[QR * c:QR * (c + 1)].T               # [HIDDEN, 512]
        wkr_c = Wkr[KRR * c:KRR * (c + 1)]             # [64, HIDDEN]
        wfk_c = Wupk[KRR * c:KRR * (c + 1)] @ Wdk      # [64, HIDDEN] fused
        wfv_c = Wupv[HEAD_DIM * c:HEAD_DIM * (c + 1)] @ Wdk  # [128, HIDDEN]
        kvrows = np.empty((128, HIDDEN), np.float32)
        kvrows[0:32] = wkr_c[0:32]     # krA: rope dims 0:32
        kvrows[32:64] = wfk_c[0:32]    # kcA: nope dims 32:64
        kvrows[64:96] = wkr_c[32:64]   # krB: rope dims 64:96
        kvrows[96:128] = wfk_c[32:64]  # kcB: nope dims 96:128
        wcomb = np.concatenate([wq_t, kvrows.T, wfv_c.T], axis=1)  # [HIDDEN, 768]
        wo_t = Wo[:, QR * c:QR * (c + 1)].T            # [512, HIDDEN]
        in_maps.append({
            "hidT": hidT,
            "wcomb": np.ascontiguousarray(wcomb).astype(bf16),
            "wo_t": np.ascontiguousarray(wo_t).astype(bf16),
            "ropes": ropes,
            "masks": masks.astype(bf16),
            "ones": np.ones((128, 1), bf16),
            "ident": np.eye(128, dtype=np.float32).astype(bf16),
        })
    return in_maps


_NC_CACHE = {}


def _get_program(key=(B, S, 512, 512)):
    if key not in _NC_CACHE:
        _NC_CACHE[key] = build_program(*key)
    return _NC_CACHE[key]


def kernel(hidden_states, Wq, Wkr, Wdk, Wupk, Wupv, Wo):
    from concourse.bass_utils import run_bass_kernel_spmd

    in_maps = make_in_maps(np.asarray(hidden_states), np.asarray(Wq),
                           np.asarray(Wkr), np.asarray(Wdk), np.asarray(Wupk),
                           np.asarray(Wupv), np.asarray(Wo))
    nc = _get_program()
    res = run_bass_kernel_spmd(nc, in_maps, list(range(NCORES)))
    out = res.results[0]["out_part"].astype(np.float32)
    for i in range(1, NCORES):
        out = out + res.results[i]["out_part"].astype(np.float32)
    return out.reshape(B, S, HIDDEN).astype(np.float32)
